# revision 1
# baseline (speedup 1.0000x reference)
"""Trainium2 Bass kernel for nn_DLP_Loss (retrieval_knn).

loss = cross_entropy(scores, target)
     + (0.5/K) * sum_i sum_{k in 5-NN same-class} mean_d (x_i - x_nbr)^2

Strategy (8 NeuronCores, SPMD):
  * Host: stable-sort rows by class. Queries are data-parallel sharded
    (1024 rows/core). Each core receives only the contiguous key window
    covering the classes its queries belong to (padded to a uniform W so
    the single SPMD program works for all cores).
  * Device: for each 128-query tile, PSUM = 2*x_i.x_j - |x_j|^2
    - BIG*(t_i - t_j)^2 via two chained matmuls (K=128 features, then a
    K=4 "mask + key-norm" matmul; the BIG terms cancel exactly for
    same-class pairs and poison different-class/pad columns). Since
    d2(i,j) = |x_i|^2 - PSUM(i,j), the row maximum is always self
    (d2=0) and the next 5 are the 5 nearest same-class neighbors: one
    DVE Max8 instruction per tile gives them with no gather.
    sum_sel d2 = cnt*|x_i|^2 - sum_sel v with |x_i|^2 = Max8 slot 0.
  * Cross-entropy for the core's rows is computed on-chip (Exp/Ln).
  * Each core writes [sum_pair_d2, sum_ce]; host adds the 8 partials.
"""

import os
import sys
import numpy as np

if "/opt/trn_rl_repo" not in sys.path:
    sys.path.insert(0, "/opt/trn_rl_repo")

import concourse.bass as bass
import concourse.bacc as bacc
import concourse.mybir as mybir
import concourse.tile as tile
from concourse import bass_utils

F32 = mybir.dt.float32
F32R = mybir.dt.float32r
BF16 = mybir.dt.bfloat16
AX = mybir.AxisListType
ALU = mybir.AluOpType
ACTF = mybir.ActivationFunctionType

N_CORES = 8
K = 5
BIG = float(2**30)
PADV = 100.0
MMDT_NAME = os.environ.get("KNN_MMDT", "bf16")  # bf16 | f32r | f32

# test.py introspection: last BassKernelResults from run_bass_kernel_spmd
LAST_RESULTS = None
_PROGRAM_CACHE = {}


def _maybe_enable_trace_hook():
    """Register the axon NTFF profile hook so BASS_TRACE=1 yields exec_time_ns.

    Harmless no-op if the boot shim is unavailable (fresh grading env)."""
    if not os.environ.get("BASS_TRACE"):
        return
    if "antenv.axon_hooks" in sys.modules:
        return
    try:
        import types

        import trn_agent_boot.trn_boot as trn_boot

        mod = types.ModuleType("antenv.axon_hooks")
        hook = [trn_boot._ntff_profile_via_ctypes("/opt/axon/libaxon_pjrt.so")]
        mod.set_axon_ntff_profile_hook = lambda h: hook.__setitem__(0, h)
        mod.get_axon_ntff_profile_hook = lambda: hook[0]
        sys.modules["antenv.axon_hooks"] = mod
    except Exception:
        pass


def _build_program(W, n_tiles):
    """One SPMD program; per-core data differs only through the input maps."""
    nch = W // 512
    nc = bacc.Bacc("TRN2", target_bir_lowering=False, debug=False,
                   num_devices=N_CORES)

    # Matmul operand dtype. bf16 moving data streams at the PE's native
    # 1 cycle/row (fp32 takes 4, fp32r ~3.4 measured); the BIG mask terms
    # are small-integer multiples of 2^30 and stay exact in bf16, and the
    # bf16 rounding of x / |x_j|^2 perturbs the loss by only a few e-6.
    MMDT = {"bf16": BF16, "f32r": F32R, "f32": F32}[MMDT_NAME]

    npc = n_tiles * 128
    d_q2t = nc.dram_tensor("q2t", (128, npc), MMDT, kind="ExternalInput")
    d_keys = nc.dram_tensor("keyst", (128, W), MMDT, kind="ExternalInput")
    d_mlhs = nc.dram_tensor("mlhst", (4, npc), MMDT, kind="ExternalInput")
    d_mrhs = nc.dram_tensor("mrhs4", (4, W), MMDT, kind="ExternalInput")
    d_scores = nc.dram_tensor("scoresr", (128, n_tiles * 7), F32,
                              kind="ExternalInput")
    d_tq = nc.dram_tensor("tqr", (128, n_tiles), F32, kind="ExternalInput")
    d_out = nc.dram_tensor("out", (1, 8), F32, kind="ExternalOutput")

    # PSUM groups of up to 1024 cols (2 banks) -> half as many Max8 calls;
    # matmuls still write 512-col (single-bank) slices.
    groups = []
    off = 0
    while off < W:
        glen = min(1024, W - off)
        sub = [(off, min(512, glen))]
        if glen > 512:
            sub.append((off + 512, glen - 512))
        groups.append((off, glen, sub))
        off += glen
    ngr = len(groups)

    with tile.TileContext(nc) as tc:
        with (
            tc.tile_pool(name="big", bufs=1) as big,
            tc.tile_pool(name="small", bufs=4) as small,
            tc.tile_pool(name="pmain", bufs=3, space=bass.MemorySpace.PSUM) as pmain,
            tc.tile_pool(name="psmall", bufs=1, space=bass.MemorySpace.PSUM) as psmall,
        ):
            keys_sb = big.tile([128, W], MMDT)
            q2t_sb = big.tile([128, npc], MMDT)
            mlhs_sb = big.tile([4, npc], MMDT)
            mrhs_sb = big.tile([4, W], MMDT)
            scores_sb = big.tile([128, n_tiles * 7], F32)
            tq_sb = big.tile([128, n_tiles], F32)
            acc5 = big.tile([128, n_tiles], F32)
            accce = big.tile([128, n_tiles], F32)
            pack2 = big.tile([128, 2], F32)
            ones128 = big.tile([128, 1], F32)
            ci32 = big.tile([128, 7], mybir.dt.int32)
            iof = big.tile([128, 7], F32)
            outsb = big.tile([1, 8], F32)

            nc.gpsimd.memset(ones128[:], 1.0)
            nc.gpsimd.iota(ci32[:], pattern=[[1, 7]], base=0,
                           channel_multiplier=0)
            nc.vector.tensor_copy(iof[:], ci32[:])

            # loads — tile-0-critical first (mask rows, first keys group),
            # split across SP and GpSimd queues so dispatch parallelizes
            nc.sync.dma_start(mrhs_sb[:], d_mrhs.ap())
            nc.sync.dma_start(mlhs_sb[:], d_mlhs.ap())
            nc.gpsimd.dma_start(q2t_sb[:], d_q2t.ap())
            for gi, (goff, glen, _sub) in enumerate(groups):
                sl = slice(goff, goff + glen)
                eng = nc.sync if gi == 0 else nc.gpsimd
                eng.dma_start(keys_sb[:, sl], d_keys.ap()[:, sl])
            nc.gpsimd.dma_start(scores_sb[:], d_scores.ap())
            nc.gpsimd.dma_start(tq_sb[:], d_tq.ap())

            # main: P[i,j] = -BIG*(t_i-t_j)^2 - |x_j|^2 + 2*x_i.x_j.
            # Max8 reads each PSUM group directly (per-group top-8 -> exact
            # global top-8 via a final Max8 over the candidates), so the
            # distance rows are never materialized in SBUF.
            o8all = big.tile([128, n_tiles * 8], F32)
            cand = big.tile([128, n_tiles * ngr * 8], F32)
            for t in range(n_tiles):
                tsl = slice(t * 128, (t + 1) * 128)
                for gi, (goff, glen, sub) in enumerate(groups):
                    pm = pmain.tile([128, 1024], F32)
                    for (coff, clen) in sub:
                        po = coff - goff
                        nc.tensor.matmul(pm[:, po:po + clen],
                                         mlhs_sb[:, tsl],
                                         mrhs_sb[:, coff:coff + clen],
                                         start=True, stop=False)
                        nc.tensor.matmul(pm[:, po:po + clen],
                                         q2t_sb[:, tsl],
                                         keys_sb[:, coff:coff + clen],
                                         start=False, stop=True)
                    c0 = (t * ngr + gi) * 8
                    v = nc.vector
                    v.add_instruction(
                        mybir.InstMax(
                            name=nc.get_next_instruction_name(),
                            ins=[v.lower_ap(pm[:, :glen])],
                            outs=[v.lower_ap(cand[:, c0:c0 + 8])],
                        )
                    )
                nc.vector.max(o8all[:, t * 8:(t + 1) * 8],
                              cand[:, t * ngr * 8:(t + 1) * ngr * 8])

            # slots 1..5 per tile = 5 nearest same-class neighbors (slot 0 =
            # self, since d2(i,i)=0 maximizes P). One batched pass over all
            # tiles — per-tile scalar chains serialize on cross-engine sems.
            o83 = o8all[:].rearrange("p (t k) -> p t k", k=8)
            v5 = o83[:, :, 1:6]
            mask5 = small.tile([128, n_tiles, 5], F32)
            nc.vector.tensor_scalar(out=mask5[:], in0=v5, scalar1=-1.0e5,
                                    scalar2=None, op0=ALU.is_gt)
            cnt = small.tile([128, n_tiles], F32)
            nc.vector.reduce_sum(cnt[:], mask5[:], axis=AX.X)
            mv = small.tile([128, n_tiles, 5], F32)
            smv = small.tile([128, n_tiles], F32)
            nc.vector.tensor_mul(mv[:], v5, mask5[:])
            nc.vector.reduce_sum(smv[:], mv[:], axis=AX.X)
            slot0 = o83[:, :, 0:1].rearrange("p t k -> p (t k)")
            c1 = small.tile([128, n_tiles], F32)
            nc.vector.tensor_mul(c1[:], cnt[:], slot0)
            nc.vector.tensor_sub(acc5[:], c1[:], smv[:])

            # cross-entropy, batched: ce = max + ln(sum exp(s - max)) - s[t]
            s3 = scores_sb[:].rearrange("p (t c) -> p t c", c=7)
            m8 = small.tile([128, n_tiles], F32)
            nc.vector.reduce_max(m8[:], s3, axis=AX.X)
            m8b = m8[:].rearrange("p (t c) -> p t c", c=1).broadcast_to(
                (128, n_tiles, 7))
            sm = small.tile([128, n_tiles, 7], F32)
            nc.vector.tensor_sub(sm[:], s3, m8b)
            e = small.tile([128, n_tiles, 7], F32)
            nc.scalar.activation(e[:].rearrange("p t c -> p (t c)"),
                                 sm[:].rearrange("p t c -> p (t c)"),
                                 ACTF.Exp)
            se = small.tile([128, n_tiles], F32)
            nc.vector.reduce_sum(se[:], e[:], axis=AX.X)
            lnse = small.tile([128, n_tiles], F32)
            nc.scalar.activation(lnse[:], se[:], ACTF.Ln)
            iof3 = iof[:].rearrange("p (t c) -> p t c", c=7).broadcast_to(
                (128, n_tiles, 7))
            tqb = tq_sb[:].rearrange("p (t c) -> p t c", c=1).broadcast_to(
                (128, n_tiles, 7))
            cmask = small.tile([128, n_tiles, 7], F32)
            nc.vector.tensor_tensor(out=cmask[:], in0=iof3, in1=tqb,
                                    op=ALU.is_equal)
            junk = small.tile([128, n_tiles, 7], F32)
            st = small.tile([128, n_tiles], F32)
            nc.vector.tensor_mul(junk[:], s3, cmask[:])
            nc.vector.reduce_sum(st[:], junk[:], axis=AX.X)
            t1 = small.tile([128, n_tiles], F32)
            nc.vector.tensor_add(t1[:], m8[:], lnse[:])
            nc.vector.tensor_sub(accce[:], t1[:], st[:])

            # fold partitions: out = [sum pair_d2, sum ce, 0...]
            nc.vector.reduce_sum(pack2[:, 0:1], acc5[:], axis=AX.X)
            nc.vector.reduce_sum(pack2[:, 1:2], accce[:], axis=AX.X)
            pf = psmall.tile([1, 2], F32)
            nc.tensor.matmul(pf[:], ones128[:], pack2[:],
                             start=True, stop=True)
            nc.gpsimd.memset(outsb[:], 0.0)
            nc.scalar.copy(outsb[0:1, 0:2], pf[:])
            nc.sync.dma_start(d_out.ap(), outsb[:])

    nc.compile()
    return nc


def _class_perm(tg):
    """Row permutation grouping rows by class. Class blocks can be laid out
    in any order; pick the order minimizing the widest per-core window
    (brute force over <=8! orders)."""
    import itertools

    n = tg.shape[0]
    npc = n // N_CORES
    nclass = int(tg.max()) + 1 if n else 1
    counts = np.bincount(tg, minlength=nclass)

    def max_span(order):
        sizes = np.array([counts[c] for c in order])
        ends = np.cumsum(sizes)
        starts = ends - sizes
        worst = 0
        for c in range(N_CORES):
            r0, r1 = c * npc, (c + 1) * npc - 1
            lo = starts[np.searchsorted(ends, r0, "right")]
            hi = ends[np.searchsorted(ends, r1, "right")]
            worst = max(worst, hi - lo)
        return worst

    best = min(itertools.permutations(range(nclass)),
               key=max_span) if nclass <= 8 else tuple(range(nclass))
    rank = np.empty(nclass, np.int64)
    for pos, c in enumerate(best):
        rank[c] = pos
    return np.argsort(rank[tg], kind="stable"), rank


def _prep_inputs(x, sc, tg):
    """Sort by class, build the 8 per-core input maps."""
    n, d = x.shape
    npc = n // N_CORES
    nclass = int(tg.max()) + 1 if n else 1
    perm, rank = _class_perm(tg)
    xs = np.ascontiguousarray(x[perm])
    ss = np.ascontiguousarray(sc[perm])
    ts = tg[perm]
    tsr = rank[ts]  # class rank, sorted ascending
    xsT = np.ascontiguousarray(xs.T)  # (128, N)

    clo = np.searchsorted(tsr, np.arange(nclass), "left")
    chi = np.searchsorted(tsr, np.arange(nclass), "right")
    row_lo = clo[tsr]
    row_hi = chi[tsr]

    spans = []
    for c in range(N_CORES):
        r0, r1 = c * npc, (c + 1) * npc - 1
        spans.append((int(row_lo[r0]), int(row_hi[r1])))
    wmax = max(hi - lo for lo, hi in spans)
    W = max(512, -(-wmax // 8) * 8)
    if 0 < W % 512 < 8:  # last chunk must satisfy Max8's free>=8
        W += 8

    tsf = ts.astype(np.float64)
    k2 = (xs.astype(np.float64) ** 2).sum(1)  # |x_j|^2 per sorted row

    if MMDT_NAME == "bf16":
        import ml_dtypes
        mm_np = ml_dtypes.bfloat16
    else:
        mm_np = np.float32

    in_maps = []
    for c in range(N_CORES):
        r0 = c * npc
        r1 = r0 + npc
        wlo, whi = spans[c]
        ww = whi - wlo

        keys = np.zeros((128, W), np.float32)
        keys[:, :ww] = xsT[:, wlo:whi]

        # pad cols: t=-1 -> penalty <= -BIG for every query class >= 0
        twin = np.full((W,), -1.0, np.float64)
        twin[:ww] = tsf[wlo:whi]
        mrhs4 = np.zeros((4, W), np.float32)
        mrhs4[0] = 1.0
        mrhs4[1] = twin
        mrhs4[2] = -BIG * twin * twin
        mrhs4[3, :ww] = -k2[wlo:whi]

        tq = tsf[r0:r1]
        mlhs = np.empty((4, npc), np.float32)
        mlhs[0] = -BIG * tq * tq
        mlhs[1] = 2.0 * BIG * tq
        mlhs[2] = 1.0
        mlhs[3] = 1.0

        in_maps.append({
            "q2t": np.ascontiguousarray(2.0 * xsT[:, r0:r1]).astype(mm_np),
            "keyst": keys.astype(mm_np),
            "mlhst": mlhs.astype(mm_np),
            "mrhs4": mrhs4.astype(mm_np),
            "scoresr": np.ascontiguousarray(
                ss[r0:r1].reshape(-1, 128, 7).transpose(1, 0, 2)
            ).reshape(128, -1),
            "tqr": np.ascontiguousarray(
                tq.reshape(-1, 128).T.astype(np.float32)),
        })
    return in_maps, W, npc // 128


def kernel(input, scores, target):
    global LAST_RESULTS
    _maybe_enable_trace_hook()

    x = np.asarray(input, np.float32)
    sc = np.asarray(scores, np.float32)
    tg = np.asarray(target).astype(np.int64)
    n, d = x.shape

    in_maps, W, n_tiles = _prep_inputs(x, sc, tg)

    key = (W, n_tiles)
    if key not in _PROGRAM_CACHE:
        _PROGRAM_CACHE[key] = _build_program(W, n_tiles)
    nc = _PROGRAM_CACHE[key]

    res = bass_utils.run_bass_kernel_spmd(
        nc, in_maps, core_ids=list(range(N_CORES)))
    LAST_RESULTS = res

    pair_d2 = 0.0
    ce_sum = 0.0
    for r in res.results:
        o = np.asarray(r["out"], np.float64).reshape(-1)
        pair_d2 += o[0]
        ce_sum += o[1]

    loss = ce_sum / n + pair_d2 * 0.5 / (K * d)
    return np.float32(loss)



# revision 2
# speedup vs baseline: 1.7105x; 1.7105x over previous
"""Trainium2 Bass kernel for nn_DLP_Loss (retrieval_knn).

loss = cross_entropy(scores, target)
     + (0.5/K) * sum_i sum_{k in 5-NN same-class} mean_d (x_i - x_nbr)^2

Strategy v2 (8 NeuronCores, SPMD, class-pure query tiles):
  * Host: stable-sort rows by class. Each 128-query tile holds queries of a
    single class (classes padded to tile multiples), so the tile's candidate
    set is exactly its class's contiguous key window -- no BIG-mask matmul
    and no multi-group Max8 merging are needed.
  * Each core runs 9 tiles; tile slot t reads key-window block B[t] of a
    fixed per-core 3-block window buffer (block capacities 4/3/2 tiles).
    A small exact search assigns classes to (core, block) pieces; all
    per-core variation lives in the DMA'd data, the program is uniform.
  * Device per tile: PSUM[128, WT] = bias(-|x_j|^2, K=2 bf16 rows for
    precision) + 2*x_i . x_j (K=128 bf16). One Max8 over the whole window
    gives slot0 = self (d2=0 is always the row max) and slots 1..5 = the
    5 nearest same-class neighbors. sum_sel d2 = cnt*slot0 - sum_sel P.
  * Pad queries / dummy tiles are killed by a per-query flag; pad key
    columns carry bias -1e9 so they never enter the top-8.
  * Cross-entropy computed on-chip from scores; per-core partial sums
    [pair_d2, ce] are DMA'd out and summed on host.
"""

import os
import sys
import numpy as np

if "/opt/trn_rl_repo" not in sys.path:
    sys.path.insert(0, "/opt/trn_rl_repo")

import concourse.bass as bass
import concourse.bacc as bacc
import concourse.mybir as mybir
import concourse.tile as tile
from concourse import bass_utils

F32 = mybir.dt.float32
BF16 = mybir.dt.bfloat16
AX = mybir.AxisListType
ALU = mybir.AluOpType
ACTF = mybir.ActivationFunctionType

N_CORES = 8
K = 5
TPC = 9                       # tiles per core
BLK_OF_TILE = [0, 0, 0, 0, 1, 1, 1, 2, 2]   # window block per tile slot
BLK_CAPS = [4, 3, 2]          # tile capacity of each window block
WT = 1240                     # window width (>= max class size, mult of 8)
NEG = -1.0e9                  # pad-column poison (must stay > -inf, < -1e5)

LAST_RESULTS = None
_PROGRAM_CACHE = {}


def _maybe_enable_trace_hook():
    """Register the axon NTFF profile hook so BASS_TRACE=1 yields exec_time_ns.

    Harmless no-op if the boot shim is unavailable (fresh grading env)."""
    if not os.environ.get("BASS_TRACE"):
        return
    if "antenv.axon_hooks" in sys.modules:
        return
    try:
        import types

        import trn_agent_boot.trn_boot as trn_boot

        mod = types.ModuleType("antenv.axon_hooks")
        hook = [trn_boot._ntff_profile_via_ctypes("/opt/axon/libaxon_pjrt.so")]
        mod.set_axon_ntff_profile_hook = lambda h: hook.__setitem__(0, h)
        mod.get_axon_ntff_profile_hook = lambda: hook[0]
        sys.modules["antenv.axon_hooks"] = mod
    except Exception:
        pass


def _build_program():
    nc = bacc.Bacc("TRN2", target_bir_lowering=False, debug=False,
                   num_devices=N_CORES)

    d_keys = nc.dram_tensor("keys3", (128, 3 * WT), BF16, kind="ExternalInput")
    d_mrhs = nc.dram_tensor("mrhs", (2, 3 * WT), BF16, kind="ExternalInput")
    d_q2t = nc.dram_tensor("q2t", (128, TPC * 128), BF16, kind="ExternalInput")
    d_scores = nc.dram_tensor("scoresr", (128, TPC * 7), F32,
                              kind="ExternalInput")
    d_tq = nc.dram_tensor("tqr", (128, TPC), F32, kind="ExternalInput")
    d_flag = nc.dram_tensor("flagq", (128, TPC), F32, kind="ExternalInput")
    d_out = nc.dram_tensor("out", (1, 8), F32, kind="ExternalOutput")

    chunks = [(0, 512), (512, 512), (1024, WT - 1024)]

    with tile.TileContext(nc) as tc:
        with (
            tc.tile_pool(name="big", bufs=1) as big,
            tc.tile_pool(name="small", bufs=4) as small,
            tc.tile_pool(name="pmain", bufs=2, space=bass.MemorySpace.PSUM) as pmain,
            tc.tile_pool(name="psmall", bufs=1, space=bass.MemorySpace.PSUM) as psmall,
        ):
            keys_sb = big.tile([128, 3 * WT], BF16)
            q2t_sb = big.tile([128, TPC * 128], BF16)
            mrhs_sb = big.tile([2, 3 * WT], BF16)
            ones2 = big.tile([2, 128], BF16)
            scores_sb = big.tile([128, TPC * 7], F32)
            tq_sb = big.tile([128, TPC], F32)
            flag_sb = big.tile([128, TPC], F32)
            o8all = big.tile([128, TPC * 8], F32)
            acc5 = big.tile([128, TPC], F32)
            accce = big.tile([128, TPC], F32)
            pack2 = big.tile([128, 2], F32)
            ones128 = big.tile([128, 1], F32)
            ci32 = big.tile([128, 7], mybir.dt.int32)
            iof = big.tile([128, 7], F32)
            outsb = big.tile([1, 8], F32)

            nc.gpsimd.memset(ones2[:], 1.0)
            nc.gpsimd.memset(ones128[:], 1.0)
            nc.gpsimd.iota(ci32[:], pattern=[[1, 7]], base=0,
                           channel_multiplier=0)
            nc.vector.tensor_copy(iof[:], ci32[:])

            # DMA: few large transfers, three queues (sync / scalar / gpsimd).
            # tile-0-critical first on each ring.
            nc.sync.dma_start(mrhs_sb[:], d_mrhs.ap())
            nc.sync.dma_start(keys_sb[:, 0:WT], d_keys.ap()[:, 0:WT])
            nc.scalar.dma_start(scores_sb[:], d_scores.ap())
            nc.scalar.dma_start(tq_sb[:], d_tq.ap())
            nc.scalar.dma_start(flag_sb[:], d_flag.ap())
            nc.scalar.dma_start(keys_sb[:, WT:2 * WT],
                                d_keys.ap()[:, WT:2 * WT])
            nc.gpsimd.dma_start(q2t_sb[:], d_q2t.ap())
            nc.gpsimd.dma_start(keys_sb[:, 2 * WT:3 * WT],
                                d_keys.ap()[:, 2 * WT:3 * WT])

            # cross-entropy first in Vector program order: it runs while the
            # first tile's matmuls are still waiting on key DMA.
            s3 = scores_sb[:].rearrange("p (t c) -> p t c", c=7)
            m8 = small.tile([128, TPC], F32)
            nc.vector.reduce_max(m8[:], s3, axis=AX.X)
            m8b = m8[:].rearrange("p (t c) -> p t c", c=1).broadcast_to(
                (128, TPC, 7))
            sm = small.tile([128, TPC, 7], F32)
            nc.vector.tensor_sub(sm[:], s3, m8b)
            e = small.tile([128, TPC, 7], F32)
            nc.scalar.activation(e[:].rearrange("p t c -> p (t c)"),
                                 sm[:].rearrange("p t c -> p (t c)"),
                                 ACTF.Exp)
            se = small.tile([128, TPC], F32)
            nc.vector.reduce_sum(se[:], e[:], axis=AX.X)
            lnse = small.tile([128, TPC], F32)
            nc.scalar.activation(lnse[:], se[:], ACTF.Ln)
            iof3 = iof[:].rearrange("p (t c) -> p t c", c=7).broadcast_to(
                (128, TPC, 7))
            tqb = tq_sb[:].rearrange("p (t c) -> p t c", c=1).broadcast_to(
                (128, TPC, 7))
            cmask = small.tile([128, TPC, 7], F32)
            nc.vector.tensor_tensor(out=cmask[:], in0=iof3, in1=tqb,
                                    op=ALU.is_equal)
            junk = small.tile([128, TPC, 7], F32)
            st = small.tile([128, TPC], F32)
            nc.vector.tensor_mul(junk[:], s3, cmask[:])
            nc.vector.reduce_sum(st[:], junk[:], axis=AX.X)
            t1 = small.tile([128, TPC], F32)
            nc.vector.tensor_add(t1[:], m8[:], lnse[:])
            t2 = small.tile([128, TPC], F32)
            nc.vector.tensor_sub(t2[:], t1[:], st[:])
            nc.vector.tensor_mul(accce[:], t2[:], flag_sb[:])

            # main loop: one PSUM window + one Max8 per tile
            for t in range(TPC):
                koff = BLK_OF_TILE[t] * WT
                qsl = slice(t * 128, (t + 1) * 128)
                pm = pmain.tile([128, WT], F32)
                for (co, cl) in chunks:
                    nc.tensor.matmul(pm[:, co:co + cl], ones2[:],
                                     mrhs_sb[:, koff + co:koff + co + cl],
                                     start=True, stop=False)
                for (co, cl) in chunks:
                    nc.tensor.matmul(pm[:, co:co + cl], q2t_sb[:, qsl],
                                     keys_sb[:, koff + co:koff + co + cl],
                                     start=False, stop=True)
                v = nc.vector
                v.add_instruction(
                    mybir.InstMax(
                        name=nc.get_next_instruction_name(),
                        ins=[v.lower_ap(pm[:, 0:WT])],
                        outs=[v.lower_ap(o8all[:, t * 8:(t + 1) * 8])],
                    )
                )

            # slots 1..5 per tile = 5 nearest same-class neighbors (slot 0 =
            # self). Batched over all tiles.
            o83 = o8all[:].rearrange("p (t k) -> p t k", k=8)
            v5 = o83[:, :, 1:6]
            mask5 = small.tile([128, TPC, 5], F32)
            nc.vector.tensor_scalar(out=mask5[:], in0=v5, scalar1=-1.0e5,
                                    scalar2=None, op0=ALU.is_gt)
            cnt = small.tile([128, TPC], F32)
            nc.vector.reduce_sum(cnt[:], mask5[:], axis=AX.X)
            mv = small.tile([128, TPC, 5], F32)
            smv = small.tile([128, TPC], F32)
            nc.vector.tensor_mul(mv[:], v5, mask5[:])
            nc.vector.reduce_sum(smv[:], mv[:], axis=AX.X)
            slot0 = o83[:, :, 0:1].rearrange("p t k -> p (t k)")
            c1 = small.tile([128, TPC], F32)
            nc.vector.tensor_mul(c1[:], cnt[:], slot0)
            c2 = small.tile([128, TPC], F32)
            nc.vector.tensor_sub(c2[:], c1[:], smv[:])
            nc.vector.tensor_mul(acc5[:], c2[:], flag_sb[:])

            # fold partitions: out = [sum pair_d2, sum ce, 0...]
            nc.vector.reduce_sum(pack2[:, 0:1], acc5[:], axis=AX.X)
            nc.vector.reduce_sum(pack2[:, 1:2], accce[:], axis=AX.X)
            pf = psmall.tile([1, 2], F32)
            nc.tensor.matmul(pf[:], ones128[:], pack2[:],
                             start=True, stop=True)
            nc.gpsimd.memset(outsb[:], 0.0)
            nc.scalar.copy(outsb[0:1, 0:2], pf[:])
            nc.sync.dma_start(d_out.ap(), outsb[:])

    nc.compile()
    return nc


def _assign_pieces(tcounts):
    """Assign each class's tiles to (core, block) pieces.

    Pieces are the per-core window blocks with capacities BLK_CAPS. Returns
    per-core, per-block: (class_id or None, [class-tile indices])."""
    nclass = len(tcounts)
    navail = {4: 0, 3: 0, 2: 0}
    for cap in BLK_CAPS:
        navail[cap] += N_CORES
    order = sorted(range(nclass), key=lambda c: -tcounts[c])

    # For each class choose piece-cap counts (n4, n3, n2) covering its tile
    # count; DFS over classes, preferring low overshoot then few pieces.
    def combos(need, avail):
        out = []
        for n4 in range(avail[4] + 1):
            for n3 in range(avail[3] + 1):
                for n2 in range(avail[2] + 1):
                    tot = 4 * n4 + 3 * n3 + 2 * n2
                    if tot >= need and tot - need <= 3:
                        out.append((tot - need, n4 + n3 + n2, n4, n3, n2))
        out.sort()
        return out

    chosen = [None] * nclass

    def dfs(i, avail):
        if i == len(order):
            return True
        c = order[i]
        for (_ov, _np, n4, n3, n2) in combos(tcounts[c], avail):
            avail2 = {4: avail[4] - n4, 3: avail[3] - n3, 2: avail[2] - n2}
            chosen[c] = (n4, n3, n2)
            if dfs(i + 1, avail2):
                return True
        chosen[c] = None
        return False

    if not dfs(0, dict(navail)):
        raise RuntimeError(f"piece assignment failed for {tcounts}")

    # map cap choices to concrete (core, blk) pieces
    free = {cap: [] for cap in (4, 3, 2)}
    for core in range(N_CORES):
        for blk, cap in enumerate(BLK_CAPS):
            free[cap].append((core, blk))
    plan = [[(None, []) for _ in BLK_CAPS] for _ in range(N_CORES)]
    for c in order:
        n4, n3, n2 = chosen[c]
        pieces = []
        for cap, npc in ((4, n4), (3, n3), (2, n2)):
            for _ in range(npc):
                pieces.append(free[cap].pop(0) + (cap,))
        ti = 0
        for (core, blk, cap) in pieces:
            take = min(cap, tcounts[c] - ti)
            plan[core][blk] = (c, list(range(ti, ti + take)))
            ti += take
        assert ti >= tcounts[c]
    return plan


def _prep_inputs(x, sc, tg):
    n, d = x.shape
    perm = np.argsort(tg, kind="stable")
    xs = np.ascontiguousarray(x[perm])
    ss = np.ascontiguousarray(sc[perm])
    ts = tg[perm]
    nclass = int(ts.max()) + 1
    clo = np.searchsorted(ts, np.arange(nclass), "left")
    chi = np.searchsorted(ts, np.arange(nclass), "right")
    widths = (chi - clo).astype(int)
    assert widths.max() <= WT, (widths.max(), WT)
    tcounts = [int(-(-w // 128)) for w in widths]
    assert sum(tcounts) <= N_CORES * TPC

    plan = _assign_pieces(tcounts)
    xsT = np.ascontiguousarray(xs.T)          # (128, N)
    k2 = (xs.astype(np.float64) ** 2).sum(1)  # |x_j|^2 per sorted row

    import ml_dtypes
    bf = ml_dtypes.bfloat16

    in_maps = []
    meta = []                                  # per core: list of 9 tile descs
    for core in range(N_CORES):
        keys3 = np.zeros((128, 3 * WT), np.float32)
        mrhs = np.zeros((2, 3 * WT), np.float32)
        mrhs[0] = NEG
        q2t = np.zeros((128, TPC * 128), np.float32)
        scoresr = np.zeros((128, TPC, 7), np.float32)
        tqr = np.zeros((128, TPC), np.float32)
        flagq = np.zeros((128, TPC), np.float32)

        # fill window blocks
        for blk in range(len(BLK_CAPS)):
            c, _tiles = plan[core][blk]
            if c is None:
                continue
            w = widths[c]
            off = blk * WT
            keys3[:, off:off + w] = xsT[:, clo[c]:chi[c]]
            bias = -k2[clo[c]:chi[c]]
            r0 = np.asarray(bias, np.float32).astype(bf).astype(np.float64)
            mrhs[0, off:off + w] = r0
            mrhs[1, off:off + w] = bias - r0

        # fill tile slots
        tiles = []
        slot = {0: 0, 1: 4, 2: 7}  # first tile slot of each block
        for blk in range(len(BLK_CAPS)):
            c, tlist = plan[core][blk]
            s0 = slot[blk]
            for j, ti in enumerate(tlist):
                t = s0 + j
                qlo = clo[c] + 128 * ti
                qn = int(min(128, chi[c] - qlo))
                q2t[:, t * 128:t * 128 + qn] = 2.0 * xsT[:, qlo:qlo + qn]
                scoresr[:qn, t, :] = ss[qlo:qlo + qn]
                tqr[:qn, t] = float(c)
                flagq[:qn, t] = 1.0
                tiles.append((t, c, int(qlo), qn))
        meta.append(tiles)

        in_maps.append({
            "keys3": keys3.astype(bf),
            "mrhs": mrhs.astype(bf),
            "q2t": q2t.astype(bf),
            "scoresr": np.ascontiguousarray(scoresr.reshape(128, TPC * 7)),
            "tqr": tqr,
            "flagq": flagq,
        })
    return in_maps, meta


def kernel(input, scores, target):
    global LAST_RESULTS
    _maybe_enable_trace_hook()

    x = np.asarray(input, np.float32)
    sc = np.asarray(scores, np.float32)
    tg = np.asarray(target).astype(np.int64)
    n, d = x.shape

    in_maps, _meta = _prep_inputs(x, sc, tg)

    if "v2" not in _PROGRAM_CACHE:
        _PROGRAM_CACHE["v2"] = _build_program()
    nc = _PROGRAM_CACHE["v2"]

    res = bass_utils.run_bass_kernel_spmd(
        nc, in_maps, core_ids=list(range(N_CORES)))
    LAST_RESULTS = res

    pair_d2 = 0.0
    ce_sum = 0.0
    for r in res.results:
        o = np.asarray(r["out"], np.float64).reshape(-1)
        pair_d2 += o[0]
        ce_sum += o[1]

    loss = ce_sum / n + pair_d2 * 0.5 / (K * d)
    return np.float32(loss)


# revision 3
# speedup vs baseline: 1.7220x; 1.0067x over previous
"""Trainium2 Bass kernel for nn_DLP_Loss (retrieval_knn).

loss = cross_entropy(scores, target)
     + (0.5/K) * sum_i sum_{k in 5-NN same-class} mean_d (x_i - x_nbr)^2

Strategy v3 (8 NeuronCores, SPMD, class-pure query tiles, fp8 DoubleRow):
  * Host: stable-sort rows by class. Each 128-query tile holds queries of a
    single class (classes padded to tile multiples), so the tile's candidate
    set is exactly its class's contiguous key window -- no BIG-mask matmul
    and no multi-group Max8 merging are needed.
  * Each core runs 9 tiles; tile slot t reads key-window block B[t] of a
    fixed per-core 3-block window buffer (block capacities 4/3/2 tiles).
    A small exact search assigns classes to (core, block) pieces; all
    per-core variation lives in the DMA'd data, the program is uniform.
  * Device per tile: PSUM[128, WT] = bias(-|x_j|^2, 4 fp8 rows r0..r3 so
    quantization error is ~0.03) + 2*x_i . x_j (K=128 as fp8 DoubleRow,
    2 K-halves of 64). All matmuls run in fp8e4m3 DoubleRow perf mode
    (0.5 cycles/output column). One Max8 over the whole window gives
    slot0 = self (d2=0 is always the row max) and slots 1..5 = the 5
    nearest same-class neighbors; sum_sel d2 = 5*slot0 - sum_sel P.
  * Pad queries / dummy tiles are killed by a per-query flag; pad key
    columns carry bias 4*(-240) ~ -960 (fp8 min) which sits far below any
    real P (>= -540 by norm bounds), so they never enter the top-8.
  * Cross-entropy computed on-chip from scores; per-core partial sums
    [pair_d2, ce] are DMA'd out and summed on host.
"""

import os
import sys
import numpy as np

if "/opt/trn_rl_repo" not in sys.path:
    sys.path.insert(0, "/opt/trn_rl_repo")

import concourse.bass as bass
import concourse.bacc as bacc
import concourse.mybir as mybir
import concourse.tile as tile
from concourse import bass_utils

F32 = mybir.dt.float32
FP8 = mybir.dt.float8e4
AX = mybir.AxisListType
ALU = mybir.AluOpType
ACTF = mybir.ActivationFunctionType
DR = mybir.MatmulPerfMode.DoubleRow

N_CORES = 8
K = 5
TPC = 9                       # tiles per core
BLK_OF_TILE = [0, 0, 0, 0, 1, 1, 1, 2, 2]   # window block per tile slot
BLK_CAPS = [4, 3, 2]          # tile capacity of each window block
WT = 1248                     # window width (>= max class size, mult of 16)
FP8MIN = -240.0               # most negative normal fp8e4m3 on TRN

LAST_RESULTS = None
_PROGRAM_CACHE = {}


def _maybe_enable_trace_hook():
    """Register the axon NTFF profile hook so BASS_TRACE=1 yields exec_time_ns.

    Harmless no-op if the boot shim is unavailable (fresh grading env)."""
    if not os.environ.get("BASS_TRACE"):
        return
    if "antenv.axon_hooks" in sys.modules:
        return
    try:
        import types

        import trn_agent_boot.trn_boot as trn_boot

        mod = types.ModuleType("antenv.axon_hooks")
        hook = [trn_boot._ntff_profile_via_ctypes("/opt/axon/libaxon_pjrt.so")]
        mod.set_axon_ntff_profile_hook = lambda h: hook.__setitem__(0, h)
        mod.get_axon_ntff_profile_hook = lambda: hook[0]
        sys.modules["antenv.axon_hooks"] = mod
    except Exception:
        pass


def _build_program():
    nc = bacc.Bacc("TRN2", target_bir_lowering=False, debug=False,
                   num_devices=N_CORES)

    d_keys = nc.dram_tensor("keys3", (64, 2, 3 * WT), FP8,
                            kind="ExternalInput")
    d_mrhs = nc.dram_tensor("mrhs", (2, 2, 3 * WT), FP8,
                            kind="ExternalInput")
    d_q2t = nc.dram_tensor("q2t", (64, 2, TPC * 128), FP8,
                           kind="ExternalInput")
    d_aux = nc.dram_tensor("aux", (128, TPC * 7 + 2 * TPC), F32,
                           kind="ExternalInput")
    d_out = nc.dram_tensor("out", (1, 8), F32, kind="ExternalOutput")

    chunks = [(0, 512), (512, 512), (1024, WT - 1024)]

    with tile.TileContext(nc) as tc:
        with (
            tc.tile_pool(name="big", bufs=1) as big,
            tc.tile_pool(name="small", bufs=4) as small,
            tc.tile_pool(name="pmain", bufs=2, space=bass.MemorySpace.PSUM) as pmain,
            tc.tile_pool(name="psmall", bufs=1, space=bass.MemorySpace.PSUM) as psmall,
        ):
            keys_sb = big.tile([64, 2, 3 * WT], FP8)
            q2t_sb = big.tile([64, 2, TPC * 128], FP8)
            mrhs_sb = big.tile([2, 2, 3 * WT], FP8)
            ones22 = big.tile([2, 2, 128], FP8)
            aux_sb = big.tile([128, TPC * 7 + 2 * TPC], F32)
            o8all = big.tile([128, TPC * 8], F32)
            acc5 = big.tile([128, TPC], F32)
            accce = big.tile([128, TPC], F32)
            pack2 = big.tile([128, 2], F32)
            ones128 = big.tile([128, 1], F32)
            ci32 = big.tile([128, 7], mybir.dt.int32)
            iof = big.tile([128, 7], F32)
            outsb = big.tile([1, 8], F32)

            scores_sb = aux_sb[:, 0:TPC * 7]
            tq_sb = aux_sb[:, TPC * 7:TPC * 8]
            flag_sb = aux_sb[:, TPC * 8:TPC * 9]

            nc.gpsimd.memset(ones22[:], 1.0)
            nc.gpsimd.memset(ones128[:], 1.0)
            nc.gpsimd.iota(ci32[:], pattern=[[1, 7]], base=0,
                           channel_multiplier=0)
            nc.vector.tensor_copy(iof[:], ci32[:])

            # DMA: few large transfers on three rings, tile-0-critical first.
            nc.scalar.dma_start(mrhs_sb[:], d_mrhs.ap())
            nc.sync.dma_start(keys_sb[:, :, 0:512], d_keys.ap()[:, :, 0:512])
            nc.sync.dma_start(keys_sb[:, :, 512:WT], d_keys.ap()[:, :, 512:WT])
            nc.scalar.dma_start(aux_sb[:], d_aux.ap())
            nc.scalar.dma_start(keys_sb[:, :, WT:2 * WT],
                                d_keys.ap()[:, :, WT:2 * WT])
            nc.gpsimd.dma_start(q2t_sb[:], d_q2t.ap())
            nc.gpsimd.dma_start(keys_sb[:, :, 2 * WT:3 * WT],
                                d_keys.ap()[:, :, 2 * WT:3 * WT])

            # cross-entropy first in Vector program order: it runs while the
            # first tile's matmuls are still waiting on key DMA.
            s3 = scores_sb.rearrange("p (t c) -> p t c", c=7)
            m8 = small.tile([128, TPC], F32)
            nc.vector.reduce_max(m8[:], s3, axis=AX.X)
            m8b = m8[:].rearrange("p (t c) -> p t c", c=1).broadcast_to(
                (128, TPC, 7))
            sm = small.tile([128, TPC, 7], F32)
            nc.vector.tensor_sub(sm[:], s3, m8b)
            e = small.tile([128, TPC, 7], F32)
            nc.scalar.activation(e[:].rearrange("p t c -> p (t c)"),
                                 sm[:].rearrange("p t c -> p (t c)"),
                                 ACTF.Exp)
            se = small.tile([128, TPC], F32)
            nc.vector.reduce_sum(se[:], e[:], axis=AX.X)
            lnse = small.tile([128, TPC], F32)
            nc.scalar.activation(lnse[:], se[:], ACTF.Ln)
            iof3 = iof[:].rearrange("p (t c) -> p t c", c=7).broadcast_to(
                (128, TPC, 7))
            tqb = tq_sb.rearrange("p (t c) -> p t c", c=1).broadcast_to(
                (128, TPC, 7))
            cmask = small.tile([128, TPC, 7], F32)
            nc.vector.tensor_tensor(out=cmask[:], in0=iof3, in1=tqb,
                                    op=ALU.is_equal)
            junk = small.tile([128, TPC, 7], F32)
            st = small.tile([128, TPC], F32)
            nc.vector.tensor_mul(junk[:], s3, cmask[:])
            nc.vector.reduce_sum(st[:], junk[:], axis=AX.X)
            t1 = small.tile([128, TPC], F32)
            nc.vector.tensor_add(t1[:], m8[:], lnse[:])
            t2 = small.tile([128, TPC], F32)
            nc.vector.tensor_sub(t2[:], t1[:], st[:])
            nc.vector.tensor_mul(accce[:], t2[:], flag_sb)

            # main loop: one PSUM window + one Max8 per tile; fp8 DoubleRow
            for t in range(TPC):
                koff = BLK_OF_TILE[t] * WT
                qsl = slice(t * 128, (t + 1) * 128)
                pm = pmain.tile([128, WT], F32)
                for (co, cl) in chunks:
                    nc.tensor.matmul(pm[:, co:co + cl], ones22[:],
                                     mrhs_sb[:, :, koff + co:koff + co + cl],
                                     start=True, stop=False, perf_mode=DR)
                for (co, cl) in chunks:
                    nc.tensor.matmul(pm[:, co:co + cl], q2t_sb[:, :, qsl],
                                     keys_sb[:, :, koff + co:koff + co + cl],
                                     start=False, stop=True, perf_mode=DR)
                v = nc.vector
                v.add_instruction(
                    mybir.InstMax(
                        name=nc.get_next_instruction_name(),
                        ins=[v.lower_ap(pm[:, 0:WT])],
                        outs=[v.lower_ap(o8all[:, t * 8:(t + 1) * 8])],
                    )
                )

            # slots 1..5 per tile = 5 nearest same-class neighbors (slot 0 =
            # self; every class has >=6 members so cnt==5 always for real
            # rows, and pad rows are killed by the flag).
            o83 = o8all[:].rearrange("p (t k) -> p t k", k=8)
            v5 = o83[:, :, 1:6]
            smv = small.tile([128, TPC], F32)
            nc.vector.reduce_sum(smv[:], v5, axis=AX.X)
            slot0 = o83[:, :, 0:1].rearrange("p t k -> p (t k)")
            c1 = small.tile([128, TPC], F32)
            nc.vector.tensor_scalar(out=c1[:], in0=slot0, scalar1=float(K),
                                    scalar2=None, op0=ALU.mult)
            c2 = small.tile([128, TPC], F32)
            nc.vector.tensor_sub(c2[:], c1[:], smv[:])
            nc.vector.tensor_mul(acc5[:], c2[:], flag_sb)

            # fold partitions: out = [sum pair_d2, sum ce, 0...]
            nc.vector.reduce_sum(pack2[:, 0:1], acc5[:], axis=AX.X)
            nc.vector.reduce_sum(pack2[:, 1:2], accce[:], axis=AX.X)
            pf = psmall.tile([1, 2], F32)
            nc.tensor.matmul(pf[:], ones128[:], pack2[:],
                             start=True, stop=True)
            nc.gpsimd.memset(outsb[:], 0.0)
            nc.scalar.copy(outsb[0:1, 0:2], pf[:])
            nc.sync.dma_start(d_out.ap(), outsb[:])

    nc.compile()
    return nc


def _assign_pieces(tcounts):
    """Assign each class's tiles to (core, block) pieces.

    Pieces are the per-core window blocks with capacities BLK_CAPS. Returns
    per-core, per-block: (class_id or None, [class-tile indices])."""
    nclass = len(tcounts)
    navail = {4: 0, 3: 0, 2: 0}
    for cap in BLK_CAPS:
        navail[cap] += N_CORES
    order = sorted(range(nclass), key=lambda c: -tcounts[c])

    def combos(need, avail):
        out = []
        for n4 in range(avail[4] + 1):
            for n3 in range(avail[3] + 1):
                for n2 in range(avail[2] + 1):
                    tot = 4 * n4 + 3 * n3 + 2 * n2
                    if tot >= need and tot - need <= 3:
                        out.append((tot - need, n4 + n3 + n2, n4, n3, n2))
        out.sort()
        return out

    chosen = [None] * nclass

    def dfs(i, avail):
        if i == len(order):
            return True
        c = order[i]
        for (_ov, _np, n4, n3, n2) in combos(tcounts[c], avail):
            avail2 = {4: avail[4] - n4, 3: avail[3] - n3, 2: avail[2] - n2}
            chosen[c] = (n4, n3, n2)
            if dfs(i + 1, avail2):
                return True
        chosen[c] = None
        return False

    if not dfs(0, dict(navail)):
        raise RuntimeError(f"piece assignment failed for {tcounts}")

    free = {cap: [] for cap in (4, 3, 2)}
    for core in range(N_CORES):
        for blk, cap in enumerate(BLK_CAPS):
            free[cap].append((core, blk))
    plan = [[(None, []) for _ in BLK_CAPS] for _ in range(N_CORES)]
    for c in order:
        n4, n3, n2 = chosen[c]
        pieces = []
        for cap, npc in ((4, n4), (3, n3), (2, n2)):
            for _ in range(npc):
                pieces.append(free[cap].pop(0) + (cap,))
        ti = 0
        for (core, blk, cap) in pieces:
            take = min(cap, tcounts[c] - ti)
            plan[core][blk] = (c, list(range(ti, ti + take)))
            ti += take
        assert ti >= tcounts[c]
    return plan


def _fp8_residual_rows(v):
    """Split v (f64) into 3 fp8 rows r0+r1+r2 ~ v with error ~1e-2."""
    import ml_dtypes
    fp8 = ml_dtypes.float8_e4m3
    r0 = np.asarray(v, np.float32).astype(fp8)
    rem = v - r0.astype(np.float64)
    r1 = np.asarray(rem, np.float32).astype(fp8)
    rem = rem - r1.astype(np.float64)
    r2 = np.asarray(rem, np.float32).astype(fp8)
    return r0, r1, r2


def _prep_inputs(x, sc, tg):
    import ml_dtypes
    fp8 = ml_dtypes.float8_e4m3

    n, d = x.shape
    perm = np.argsort(tg, kind="stable")
    xs = np.ascontiguousarray(x[perm])
    ss = np.ascontiguousarray(sc[perm])
    ts = tg[perm]
    nclass = int(ts.max()) + 1
    clo = np.searchsorted(ts, np.arange(nclass), "left")
    chi = np.searchsorted(ts, np.arange(nclass), "right")
    widths = (chi - clo).astype(int)
    assert widths.max() <= WT, (widths.max(), WT)
    assert widths.min() >= K + 1, widths.min()
    tcounts = [int(-(-w // 128)) for w in widths]
    assert sum(tcounts) <= N_CORES * TPC

    plan = _assign_pieces(tcounts)
    xsT = np.ascontiguousarray(xs.T)          # (128, N)
    k2 = (xs.astype(np.float64) ** 2).sum(1)  # |x_j|^2 per sorted row
    # pad-column poison must sit below any real P = 2 x.x_j - |x_j|^2
    assert 4 * FP8MIN < -(3.0 * k2.max()) - 50.0, k2.max()

    in_maps = []
    meta = []                                  # per core: list of tile descs
    for core in range(N_CORES):
        keys3 = np.zeros((64, 2, 3 * WT), np.float32)
        mrhs = np.full((2, 2, 3 * WT), FP8MIN, np.float32)
        q2t = np.zeros((64, 2, TPC * 128), np.float32)
        aux = np.zeros((128, TPC * 7 + 2 * TPC), np.float32)
        scoresr = aux[:, :TPC * 7].reshape(128, TPC, 7)
        tqr = aux[:, TPC * 7:TPC * 8]
        flagq = aux[:, TPC * 8:TPC * 9]

        for blk in range(len(BLK_CAPS)):
            c, _tiles = plan[core][blk]
            if c is None:
                continue
            w = widths[c]
            off = blk * WT
            win = xsT[:, clo[c]:chi[c]]                    # (128, w)
            keys3[:, 0, off:off + w] = win[0:64]
            keys3[:, 1, off:off + w] = win[64:128]
            r0, r1, r2 = _fp8_residual_rows(-k2[clo[c]:chi[c]])
            mrhs[0, 0, off:off + w] = r0.astype(np.float32)
            mrhs[0, 1, off:off + w] = r1.astype(np.float32)
            mrhs[1, 0, off:off + w] = r2.astype(np.float32)
            mrhs[1, 1, off:off + w] = 0.0

        tiles = []
        slot = {0: 0, 1: 4, 2: 7}  # first tile slot of each block
        for blk in range(len(BLK_CAPS)):
            c, tlist = plan[core][blk]
            s0 = slot[blk]
            for j, ti in enumerate(tlist):
                t = s0 + j
                qlo = clo[c] + 128 * ti
                qn = int(min(128, chi[c] - qlo))
                qw = 2.0 * xsT[:, qlo:qlo + qn]
                q2t[:, 0, t * 128:t * 128 + qn] = qw[0:64]
                q2t[:, 1, t * 128:t * 128 + qn] = qw[64:128]
                scoresr[:qn, t, :] = ss[qlo:qlo + qn]
                tqr[:qn, t] = float(c)
                flagq[:qn, t] = 1.0
                tiles.append((t, c, int(qlo), qn))
        meta.append(tiles)

        in_maps.append({
            "keys3": keys3.astype(fp8),
            "mrhs": mrhs.astype(fp8),
            "q2t": q2t.astype(fp8),
            "aux": np.ascontiguousarray(aux),
        })
    return in_maps, meta


def kernel(input, scores, target):
    global LAST_RESULTS
    _maybe_enable_trace_hook()

    x = np.asarray(input, np.float32)
    sc = np.asarray(scores, np.float32)
    tg = np.asarray(target).astype(np.int64)
    n, d = x.shape

    in_maps, _meta = _prep_inputs(x, sc, tg)

    if "v3" not in _PROGRAM_CACHE:
        _PROGRAM_CACHE["v3"] = _build_program()
    nc = _PROGRAM_CACHE["v3"]

    res = bass_utils.run_bass_kernel_spmd(
        nc, in_maps, core_ids=list(range(N_CORES)))
    LAST_RESULTS = res

    pair_d2 = 0.0
    ce_sum = 0.0
    for r in res.results:
        o = np.asarray(r["out"], np.float64).reshape(-1)
        pair_d2 += o[0]
        ce_sum += o[1]

    loss = ce_sum / n + pair_d2 * 0.5 / (K * d)
    return np.float32(loss)


# revision 11
# speedup vs baseline: 1.8549x; 1.0772x over previous
"""Trainium2 Bass kernel for nn_DLP_Loss (retrieval_knn).

loss = cross_entropy(scores, target)
     + (0.5/K) * sum_i sum_{k in 5-NN same-class} mean_d (x_i - x_nbr)^2

Strategy v3 (8 NeuronCores, SPMD, class-pure query tiles, fp8 DoubleRow):
  * Host: stable-sort rows by class. Each 128-query tile holds queries of a
    single class (classes padded to tile multiples), so the tile's candidate
    set is exactly its class's contiguous key window -- no BIG-mask matmul
    and no multi-group Max8 merging are needed.
  * Each core runs 9 tiles; tile slot t reads key-window block B[t] of a
    fixed per-core 3-block window buffer (block capacities 4/3/2 tiles).
    A small exact search assigns classes to (core, block) pieces; all
    per-core variation lives in the DMA'd data, the program is uniform.
  * Device per tile: ONE fp8e4m3 DoubleRow matmul pass computes
    PSUM[128, WT] = 2*x_i . x_j - |x_j|^2 directly: DoubleRow virtualizes
    the contraction to 2x66 rows, so the 128 features ride as 64 partition
    pairs and partition 64 carries the bias -|x_j|^2 as a 2-term fp8
    residual decomposition (r0+r1, error ~0.5) against an all-ones query
    row (partition 65 is zero padding). One Max8 over the whole window
    gives slot0 = self (d2=0 is always the row max) and slots 1..5 = the
    5 nearest same-class neighbors; sum_sel d2 = 5*slot0 - sum_sel P.
  * Pad queries / dummy tiles are killed by a per-query flag; pad key
    columns carry bias 2*(-240) = -480 (fp8 min), far below the row's
    top-6 P values (~ -150 worst case), so they never enter the top-8.
  * Cross-entropy computed on-chip from scores; per-core partial sums
    [pair_d2, ce] are DMA'd out and summed on host.
"""

import os
import sys
import numpy as np

if "/opt/trn_rl_repo" not in sys.path:
    sys.path.insert(0, "/opt/trn_rl_repo")

import concourse.bass as bass
import concourse.bacc as bacc
import concourse.mybir as mybir
import concourse.tile as tile
from concourse import bass_utils

F32 = mybir.dt.float32
FP8 = mybir.dt.float8e4
AX = mybir.AxisListType
ALU = mybir.AluOpType
ACTF = mybir.ActivationFunctionType
DR = mybir.MatmulPerfMode.DoubleRow

N_CORES = 8
K = 5
TPC = 9                       # tiles per core
BLK_OF_TILE = [0, 0, 0, 0, 1, 1, 1, 2, 2]   # window block per tile slot
BLK_CAPS = [4, 3, 2]          # tile capacity of each window block
WT = 1248                     # window width (>= max class size, mult of 16)
FP8MIN = -240.0               # most negative normal fp8e4m3 on TRN

LAST_RESULTS = None
_PROGRAM_CACHE = {}


def _maybe_enable_trace_hook():
    """Register the axon NTFF profile hook so BASS_TRACE=1 yields exec_time_ns.

    Harmless no-op if the boot shim is unavailable (fresh grading env)."""
    if not os.environ.get("BASS_TRACE"):
        return
    if "antenv.axon_hooks" in sys.modules:
        return
    try:
        import types

        import trn_agent_boot.trn_boot as trn_boot

        mod = types.ModuleType("antenv.axon_hooks")
        hook = [trn_boot._ntff_profile_via_ctypes("/opt/axon/libaxon_pjrt.so")]
        mod.set_axon_ntff_profile_hook = lambda h: hook.__setitem__(0, h)
        mod.get_axon_ntff_profile_hook = lambda: hook[0]
        sys.modules["antenv.axon_hooks"] = mod
    except Exception:
        pass


def _build_program():
    nc = bacc.Bacc("TRN2", target_bir_lowering=False, debug=False,
                   num_devices=N_CORES)

    d_keys = nc.dram_tensor("keys3", (3, 66, 2, WT), FP8,
                            kind="ExternalInput")
    d_q2t = nc.dram_tensor("q2t", (66, 2, TPC * 128), FP8,
                           kind="ExternalInput")
    d_aux = nc.dram_tensor("aux", (128, TPC * 7 + 2 * TPC), F32,
                           kind="ExternalInput")
    d_out = nc.dram_tensor("out", (1, 8), F32, kind="ExternalOutput")

    chunks = [(0, 512), (512, 512), (1024, WT - 1024)]

    with tile.TileContext(nc) as tc:
        with (
            tc.tile_pool(name="big", bufs=1) as big,
            tc.tile_pool(name="small", bufs=4) as small,
            tc.tile_pool(name="pmain", bufs=2, space=bass.MemorySpace.PSUM) as pmain,
            tc.tile_pool(name="psmall", bufs=1, space=bass.MemorySpace.PSUM) as psmall,
        ):
            kwin = [big.tile([66, 2, WT], FP8, name=f"kwin{i}")
                    for i in range(3)]
            q2t_sb = big.tile([66, 2, TPC * 128], FP8)
            aux_sb = big.tile([128, TPC * 7 + 2 * TPC], F32)
            o8all = big.tile([128, TPC * 8], F32)
            acc5 = big.tile([128, TPC], F32)
            accce = big.tile([128, TPC], F32)
            pack2 = big.tile([128, 2], F32)
            ones128 = big.tile([128, 1], F32)
            ci32 = big.tile([128, 7], mybir.dt.int32)
            iof = big.tile([128, 7], F32)
            outsb = big.tile([1, 8], F32)

            scores_sb = aux_sb[:, 0:TPC * 7]
            tq_sb = aux_sb[:, TPC * 7:TPC * 8]
            flag_sb = aux_sb[:, TPC * 8:TPC * 9]

            nc.gpsimd.memset(ones128[:], 1.0)
            nc.gpsimd.iota(ci32[:], pattern=[[1, 7]], base=0,
                           channel_multiplier=0)
            nc.vector.tensor_copy(iof[:], ci32[:])

            # DMA: few large transfers on three rings, tile-0-critical first.
            nc.sync.dma_start(kwin[0][:], d_keys.ap()[0])
            nc.scalar.dma_start(aux_sb[:], d_aux.ap())
            nc.scalar.dma_start(kwin[1][:], d_keys.ap()[1])
            nc.gpsimd.dma_start(q2t_sb[:], d_q2t.ap())
            nc.gpsimd.dma_start(kwin[2][:], d_keys.ap()[2])

            # cross-entropy first in Vector program order: it runs while the
            # first tile's matmuls are still waiting on key DMA.
            s3 = scores_sb.rearrange("p (t c) -> p t c", c=7)
            m8 = small.tile([128, TPC], F32)
            nc.vector.reduce_max(m8[:], s3, axis=AX.X)
            m8b = m8[:].rearrange("p (t c) -> p t c", c=1).broadcast_to(
                (128, TPC, 7))
            sm = small.tile([128, TPC, 7], F32)
            nc.vector.tensor_sub(sm[:], s3, m8b)
            e = small.tile([128, TPC, 7], F32)
            nc.scalar.activation(e[:].rearrange("p t c -> p (t c)"),
                                 sm[:].rearrange("p t c -> p (t c)"),
                                 ACTF.Exp)
            se = small.tile([128, TPC], F32)
            nc.vector.reduce_sum(se[:], e[:], axis=AX.X)
            lnse = small.tile([128, TPC], F32)
            nc.scalar.activation(lnse[:], se[:], ACTF.Ln)
            iof3 = iof[:].rearrange("p (t c) -> p t c", c=7).broadcast_to(
                (128, TPC, 7))
            tqb = tq_sb.rearrange("p (t c) -> p t c", c=1).broadcast_to(
                (128, TPC, 7))
            cmask = small.tile([128, TPC, 7], F32)
            nc.vector.tensor_tensor(out=cmask[:], in0=iof3, in1=tqb,
                                    op=ALU.is_equal)
            junk = small.tile([128, TPC, 7], F32)
            st = small.tile([128, TPC], F32)
            nc.vector.tensor_mul(junk[:], s3, cmask[:])
            nc.vector.reduce_sum(st[:], junk[:], axis=AX.X)
            t1 = small.tile([128, TPC], F32)
            nc.vector.tensor_add(t1[:], m8[:], lnse[:])
            t2 = small.tile([128, TPC], F32)
            nc.vector.tensor_sub(t2[:], t1[:], st[:])
            nc.vector.tensor_mul(accce[:], t2[:], flag_sb)

            # main loop: one fused matmul pass + one Max8 per tile
            for t in range(TPC):
                kb = kwin[BLK_OF_TILE[t]]
                qsl = slice(t * 128, (t + 1) * 128)
                pm = pmain.tile([128, WT], F32)
                for (co, cl) in chunks:
                    nc.tensor.matmul(pm[:, co:co + cl], q2t_sb[:, :, qsl],
                                     kb[:, :, co:co + cl],
                                     start=True, stop=True, perf_mode=DR)
                v = nc.vector
                v.add_instruction(
                    mybir.InstMax(
                        name=nc.get_next_instruction_name(),
                        ins=[v.lower_ap(pm[:, 0:WT])],
                        outs=[v.lower_ap(o8all[:, t * 8:(t + 1) * 8])],
                    )
                )

            # slots 1..5 per tile = 5 nearest same-class neighbors (slot 0 =
            # self; every class has >=6 members so cnt==5 always for real
            # rows, and pad rows are killed by the flag).
            o83 = o8all[:].rearrange("p (t k) -> p t k", k=8)
            v5 = o83[:, :, 1:6]
            smv = small.tile([128, TPC], F32)
            nc.vector.reduce_sum(smv[:], v5, axis=AX.X)
            slot0 = o83[:, :, 0:1].rearrange("p t k -> p (t k)")
            c1 = small.tile([128, TPC], F32)
            nc.vector.tensor_scalar(out=c1[:], in0=slot0, scalar1=float(K),
                                    scalar2=None, op0=ALU.mult)
            c2 = small.tile([128, TPC], F32)
            nc.vector.tensor_sub(c2[:], c1[:], smv[:])
            nc.vector.tensor_mul(acc5[:], c2[:], flag_sb)

            # fold partitions: out = [sum pair_d2, sum ce, 0...]
            nc.vector.reduce_sum(pack2[:, 0:1], acc5[:], axis=AX.X)
            nc.vector.reduce_sum(pack2[:, 1:2], accce[:], axis=AX.X)
            pf = psmall.tile([1, 2], F32)
            nc.tensor.matmul(pf[:], ones128[:], pack2[:],
                             start=True, stop=True)
            nc.gpsimd.memset(outsb[:], 0.0)
            nc.scalar.copy(outsb[0:1, 0:2], pf[:])
            nc.sync.dma_start(d_out.ap(), outsb[:])

    nc.compile()
    return nc


def _assign_pieces(tcounts):
    """Assign each class's tiles to (core, block) pieces.

    Pieces are the per-core window blocks with capacities BLK_CAPS. Returns
    per-core, per-block: (class_id or None, [class-tile indices])."""
    nclass = len(tcounts)
    navail = {4: 0, 3: 0, 2: 0}
    for cap in BLK_CAPS:
        navail[cap] += N_CORES
    order = sorted(range(nclass), key=lambda c: -tcounts[c])

    def combos(need, avail):
        out = []
        for n4 in range(avail[4] + 1):
            for n3 in range(avail[3] + 1):
                for n2 in range(avail[2] + 1):
                    tot = 4 * n4 + 3 * n3 + 2 * n2
                    if tot >= need and tot - need <= 3:
                        out.append((tot - need, n4 + n3 + n2, n4, n3, n2))
        out.sort()
        return out

    chosen = [None] * nclass

    def dfs(i, avail):
        if i == len(order):
            return True
        c = order[i]
        for (_ov, _np, n4, n3, n2) in combos(tcounts[c], avail):
            avail2 = {4: avail[4] - n4, 3: avail[3] - n3, 2: avail[2] - n2}
            chosen[c] = (n4, n3, n2)
            if dfs(i + 1, avail2):
                return True
        chosen[c] = None
        return False

    if not dfs(0, dict(navail)):
        raise RuntimeError(f"piece assignment failed for {tcounts}")

    free = {cap: [] for cap in (4, 3, 2)}
    for core in range(N_CORES):
        for blk, cap in enumerate(BLK_CAPS):
            free[cap].append((core, blk))
    plan = [[(None, []) for _ in BLK_CAPS] for _ in range(N_CORES)]
    for c in order:
        n4, n3, n2 = chosen[c]
        pieces = []
        for cap, npc in ((4, n4), (3, n3), (2, n2)):
            for _ in range(npc):
                pieces.append(free[cap].pop(0) + (cap,))
        ti = 0
        for (core, blk, cap) in pieces:
            take = min(cap, tcounts[c] - ti)
            plan[core][blk] = (c, list(range(ti, ti + take)))
            ti += take
        assert ti >= tcounts[c]
    return plan


def _fp8_residual_rows(v):
    """Split v (f64) into 2 fp8 rows r0+r1 ~ v with error ~0.5."""
    import ml_dtypes
    fp8 = ml_dtypes.float8_e4m3
    r0 = np.asarray(v, np.float32).astype(fp8)
    rem = v - r0.astype(np.float64)
    r1 = np.asarray(rem, np.float32).astype(fp8)
    return r0, r1


def _prep_inputs(x, sc, tg):
    import ml_dtypes
    fp8 = ml_dtypes.float8_e4m3

    n, d = x.shape
    perm = np.argsort(tg, kind="stable")
    xs = np.ascontiguousarray(x[perm])
    ss = np.ascontiguousarray(sc[perm])
    ts = tg[perm]
    nclass = int(ts.max()) + 1
    clo = np.searchsorted(ts, np.arange(nclass), "left")
    chi = np.searchsorted(ts, np.arange(nclass), "right")
    widths = (chi - clo).astype(int)
    assert widths.max() <= WT, (widths.max(), WT)
    assert widths.min() >= K + 1, widths.min()
    tcounts = [int(-(-w // 128)) for w in widths]
    assert sum(tcounts) <= N_CORES * TPC

    plan = _assign_pieces(tcounts)
    xsT = np.ascontiguousarray(xs.T)          # (128, N)
    k2 = (xs.astype(np.float64) ** 2).sum(1)  # |x_j|^2 per sorted row
    # pad-column poison (2*FP8MIN) must sit below the row's top-6 P values,
    # which are >= |x_i|^2 - d2(5th NN) >= -max|x|^2 comfortably.
    assert 2 * FP8MIN < -(1.5 * k2.max()) - 50.0, k2.max()

    in_maps = []
    meta = []                                  # per core: list of tile descs
    for core in range(N_CORES):
        keys3 = np.zeros((3, 66, 2, WT), np.float32)
        keys3[:, 64, :, :] = FP8MIN            # bias rows: pad-col poison
        q2t = np.zeros((66, 2, TPC * 128), np.float32)
        q2t[64, :, :] = 1.0                    # all-ones row against bias
        aux = np.zeros((128, TPC * 7 + 2 * TPC), np.float32)
        scoresr = aux[:, :TPC * 7].reshape(128, TPC, 7)
        tqr = aux[:, TPC * 7:TPC * 8]
        flagq = aux[:, TPC * 8:TPC * 9]

        for blk in range(len(BLK_CAPS)):
            c, _tiles = plan[core][blk]
            if c is None:
                continue
            w = widths[c]
            win = xsT[:, clo[c]:chi[c]]                    # (128, w)
            keys3[blk, 0:64, 0, 0:w] = win[0:64]
            keys3[blk, 0:64, 1, 0:w] = win[64:128]
            r0, r1 = _fp8_residual_rows(-k2[clo[c]:chi[c]])
            keys3[blk, 64, 0, 0:w] = r0.astype(np.float32)
            keys3[blk, 64, 1, 0:w] = r1.astype(np.float32)

        tiles = []
        slot = {0: 0, 1: 4, 2: 7}  # first tile slot of each block
        for blk in range(len(BLK_CAPS)):
            c, tlist = plan[core][blk]
            s0 = slot[blk]
            for j, ti in enumerate(tlist):
                t = s0 + j
                qlo = clo[c] + 128 * ti
                qn = int(min(128, chi[c] - qlo))
                qw = 2.0 * xsT[:, qlo:qlo + qn]
                q2t[0:64, 0, t * 128:t * 128 + qn] = qw[0:64]
                q2t[0:64, 1, t * 128:t * 128 + qn] = qw[64:128]
                scoresr[:qn, t, :] = ss[qlo:qlo + qn]
                tqr[:qn, t] = float(c)
                flagq[:qn, t] = 1.0
                tiles.append((t, c, int(qlo), qn))
        meta.append(tiles)

        in_maps.append({
            "keys3": keys3.astype(fp8),
            "q2t": q2t.astype(fp8),
            "aux": np.ascontiguousarray(aux),
        })
    return in_maps, meta


def kernel(input, scores, target):
    global LAST_RESULTS
    _maybe_enable_trace_hook()

    x = np.asarray(input, np.float32)
    sc = np.asarray(scores, np.float32)
    tg = np.asarray(target).astype(np.int64)
    n, d = x.shape

    in_maps, _meta = _prep_inputs(x, sc, tg)

    if "v4" not in _PROGRAM_CACHE:
        _PROGRAM_CACHE["v4"] = _build_program()
    nc = _PROGRAM_CACHE["v4"]

    res = bass_utils.run_bass_kernel_spmd(
        nc, in_maps, core_ids=list(range(N_CORES)))
    LAST_RESULTS = res

    pair_d2 = 0.0
    ce_sum = 0.0
    for r in res.results:
        o = np.asarray(r["out"], np.float64).reshape(-1)
        pair_d2 += o[0]
        ce_sum += o[1]

    loss = ce_sum / n + pair_d2 * 0.5 / (K * d)
    return np.float32(loss)


# revision 19
# speedup vs baseline: 2.2425x; 1.2090x over previous
"""Trainium2 Bass kernel for nn_DLP_Loss (retrieval_knn).

loss = cross_entropy(scores, target)
     + (0.5/K) * sum_i sum_{k in 5-NN same-class} mean_d (x_i - x_nbr)^2

Strategy v3 (8 NeuronCores, SPMD, class-pure query tiles, fp8 DoubleRow):
  * Host: stable-sort rows by class. Each 128-query tile holds queries of a
    single class (classes padded to tile multiples), so the tile's candidate
    set is exactly its class's contiguous key window -- no BIG-mask matmul
    and no multi-group Max8 merging are needed.
  * Each core runs 9 tiles; tile slot t reads key-window block B[t] of a
    fixed per-core 3-block window buffer (block capacities 4/3/2 tiles).
    A small exact search assigns classes to (core, block) pieces; all
    per-core variation lives in the DMA'd data, the program is uniform.
  * Device per tile: ONE fp8e4m3 DoubleRow matmul pass computes
    PSUM[128, WT] = 2*x_i . x_j - |x_j|^2 directly: DoubleRow virtualizes
    the contraction to 2x66 rows, so the 128 features ride as 64 partition
    pairs and partition 64 carries the bias -|x_j|^2 as a 2-term fp8
    residual decomposition (r0+r1, error ~0.5) against an all-ones query
    row (partition 65 is zero padding). One Max8 over the whole window
    gives slot0 = self (d2=0 is always the row max) and slots 1..5 = the
    5 nearest same-class neighbors; sum_sel d2 = 5*slot0 - sum_sel P.
  * Pad queries / dummy tiles are killed by a per-query flag; pad key
    columns carry bias 2*(-240) = -480 (fp8 min), far below the row's
    top-6 P values (~ -150 worst case), so they never enter the top-8.
  * Cross-entropy computed on-chip from scores; per-core partial sums
    [pair_d2, ce] are DMA'd out and summed on host.
"""

import os
import sys
import numpy as np

if "/opt/trn_rl_repo" not in sys.path:
    sys.path.insert(0, "/opt/trn_rl_repo")

import concourse.bass as bass
import concourse.bacc as bacc
import concourse.mybir as mybir
import concourse.tile as tile
from concourse import bass_utils

F32 = mybir.dt.float32
FP8 = mybir.dt.float8e4
AX = mybir.AxisListType
ALU = mybir.AluOpType
ACTF = mybir.ActivationFunctionType
DR = mybir.MatmulPerfMode.DoubleRow

N_CORES = 8
K = 5
TPC = 9                       # tiles per core
BLK_OF_TILE = [0, 0, 0, 0, 1, 1, 1, 2, 2]   # window block per tile slot
BLK_CAPS = [4, 3, 2]          # tile capacity of each window block
WT = 1248                     # window width (>= max class size, mult of 16)
FP8MIN = -240.0               # most negative normal fp8e4m3 on TRN

LAST_RESULTS = None
_PROGRAM_CACHE = {}


def _maybe_enable_trace_hook():
    """Register the axon NTFF profile hook so BASS_TRACE=1 yields exec_time_ns.

    Harmless no-op if the boot shim is unavailable (fresh grading env)."""
    if not os.environ.get("BASS_TRACE"):
        return
    if "antenv.axon_hooks" in sys.modules:
        return
    try:
        import types

        import trn_agent_boot.trn_boot as trn_boot

        mod = types.ModuleType("antenv.axon_hooks")
        hook = [trn_boot._ntff_profile_via_ctypes("/opt/axon/libaxon_pjrt.so")]
        mod.set_axon_ntff_profile_hook = lambda h: hook.__setitem__(0, h)
        mod.get_axon_ntff_profile_hook = lambda: hook[0]
        sys.modules["antenv.axon_hooks"] = mod
    except Exception:
        pass


def _build_program(wblk):
    """wblk: per-window-block compute width (max real class width), <= WT."""
    nc = bacc.Bacc("TRN2", target_bir_lowering=False, debug=False,
                   num_devices=N_CORES)

    d_keys = nc.dram_tensor("keys3", (3, 66, 2, WT), FP8,
                            kind="ExternalInput")
    d_q2t = nc.dram_tensor("q2t", (66, 2, TPC * 128), FP8,
                           kind="ExternalInput")
    d_aux = nc.dram_tensor("aux", (128, TPC * 7 + 2 * TPC), F32,
                           kind="ExternalInput")
    d_out = nc.dram_tensor("out", (1, 8), F32, kind="ExternalOutput")

    def chunks_of(w):
        out = []
        off = 0
        while off < w:
            out.append((off, min(512, w - off)))
            off += 512
        return out

    with tile.TileContext(nc) as tc:
        with (
            tc.tile_pool(name="big", bufs=1) as big,
            tc.tile_pool(name="small", bufs=4) as small,
            tc.tile_pool(name="pmain", bufs=2, space=bass.MemorySpace.PSUM) as pmain,
            tc.tile_pool(name="psmall", bufs=1, space=bass.MemorySpace.PSUM) as psmall,
        ):
            kwin = [big.tile([66, 2, WT], FP8, name=f"kwin{i}")
                    for i in range(3)]
            q2t_sb = big.tile([66, 2, TPC * 128], FP8)
            aux_sb = big.tile([128, TPC * 7 + 2 * TPC], F32)
            o8all = big.tile([128, TPC * 8], F32)
            acc5 = big.tile([128, TPC], F32)
            accce = big.tile([128, TPC], F32)
            pack2 = big.tile([128, 2], F32)
            ones128 = big.tile([128, 1], F32)
            ci32 = big.tile([128, 7], mybir.dt.int32)
            iof = big.tile([128, 7], F32)
            outsb = big.tile([1, 8], F32)

            scores_sb = aux_sb[:, 0:TPC * 7]
            tq_sb = aux_sb[:, TPC * 7:TPC * 8]
            flag_sb = aux_sb[:, TPC * 8:TPC * 9]

            nc.gpsimd.memset(ones128[:], 1.0)
            nc.gpsimd.iota(ci32[:], pattern=[[1, 7]], base=0,
                           channel_multiplier=0)
            nc.vector.tensor_copy(iof[:], ci32[:])

            # DMA: few large transfers on three rings, tile-0-critical first.
            nc.sync.dma_start(kwin[0][:], d_keys.ap()[0])
            nc.scalar.dma_start(q2t_sb[:, :, 0:256], d_q2t.ap()[:, :, 0:256])
            nc.scalar.dma_start(aux_sb[:], d_aux.ap())
            nc.scalar.dma_start(kwin[1][:], d_keys.ap()[1])
            nc.sync.dma_start(q2t_sb[:, :, 256:TPC * 128],
                              d_q2t.ap()[:, :, 256:TPC * 128])
            nc.gpsimd.dma_start(kwin[2][:], d_keys.ap()[2])

            # cross-entropy first in Vector program order: it runs while the
            # first tile's matmuls are still waiting on key DMA.
            s3 = scores_sb.rearrange("p (t c) -> p t c", c=7)
            m8 = small.tile([128, TPC], F32)
            nc.vector.reduce_max(m8[:], s3, axis=AX.X)
            m8b = m8[:].rearrange("p (t c) -> p t c", c=1).broadcast_to(
                (128, TPC, 7))
            sm = small.tile([128, TPC, 7], F32)
            nc.vector.tensor_sub(sm[:], s3, m8b)
            e = small.tile([128, TPC, 7], F32)
            nc.scalar.activation(e[:].rearrange("p t c -> p (t c)"),
                                 sm[:].rearrange("p t c -> p (t c)"),
                                 ACTF.Exp)
            se = small.tile([128, TPC], F32)
            nc.vector.reduce_sum(se[:], e[:], axis=AX.X)
            lnse = small.tile([128, TPC], F32)
            nc.scalar.activation(lnse[:], se[:], ACTF.Ln)
            iof3 = iof[:].rearrange("p (t c) -> p t c", c=7).broadcast_to(
                (128, TPC, 7))
            tqb = tq_sb.rearrange("p (t c) -> p t c", c=1).broadcast_to(
                (128, TPC, 7))
            cmask = small.tile([128, TPC, 7], F32)
            nc.vector.tensor_tensor(out=cmask[:], in0=iof3, in1=tqb,
                                    op=ALU.is_equal)
            junk = small.tile([128, TPC, 7], F32)
            st = small.tile([128, TPC], F32)
            nc.vector.tensor_mul(junk[:], s3, cmask[:])
            nc.vector.reduce_sum(st[:], junk[:], axis=AX.X)
            t1 = small.tile([128, TPC], F32)
            nc.vector.tensor_add(t1[:], m8[:], lnse[:])
            t2 = small.tile([128, TPC], F32)
            nc.vector.tensor_sub(t2[:], t1[:], st[:])
            nc.vector.tensor_mul(accce[:], t2[:], flag_sb)

            # main loop: one fused matmul pass + one Max8 per tile
            for t in range(TPC):
                b = BLK_OF_TILE[t]
                kb = kwin[b]
                w = wblk[b]
                qsl = slice(t * 128, (t + 1) * 128)
                pm = pmain.tile([128, WT], F32, name="pm")
                for (co, cl) in chunks_of(w):
                    nc.tensor.matmul(pm[:, co:co + cl], q2t_sb[:, :, qsl],
                                     kb[:, :, co:co + cl],
                                     start=True, stop=True, perf_mode=DR)
                v = nc.vector
                v.add_instruction(
                    mybir.InstMax(
                        name=nc.get_next_instruction_name(),
                        ins=[v.lower_ap(pm[:, 0:w])],
                        outs=[v.lower_ap(o8all[:, t * 8:(t + 1) * 8])],
                    )
                )

            # slots 1..5 per tile = 5 nearest same-class neighbors (slot 0 =
            # self; every class has >=6 members so cnt==5 always for real
            # rows, and pad rows are killed by the flag).
            o83 = o8all[:].rearrange("p (t k) -> p t k", k=8)
            v5 = o83[:, :, 1:6]
            smv = small.tile([128, TPC], F32)
            nc.vector.reduce_sum(smv[:], v5, axis=AX.X)
            slot0 = o83[:, :, 0:1].rearrange("p t k -> p (t k)")
            c1 = small.tile([128, TPC], F32)
            nc.vector.tensor_scalar(out=c1[:], in0=slot0, scalar1=float(K),
                                    scalar2=None, op0=ALU.mult)
            c2 = small.tile([128, TPC], F32)
            nc.vector.tensor_sub(c2[:], c1[:], smv[:])
            nc.vector.tensor_mul(acc5[:], c2[:], flag_sb)

            # fold partitions: out = [sum pair_d2, sum ce, 0...]
            nc.vector.reduce_sum(pack2[:, 0:1], acc5[:], axis=AX.X)
            nc.vector.reduce_sum(pack2[:, 1:2], accce[:], axis=AX.X)
            pf = psmall.tile([1, 2], F32)
            nc.tensor.matmul(pf[:], ones128[:], pack2[:],
                             start=True, stop=True)
            nc.gpsimd.memset(outsb[:], 0.0)
            nc.scalar.copy(outsb[0:1, 0:2], pf[:])
            nc.sync.dma_start(d_out.ap(), outsb[:])

    nc.compile()
    return nc


def _assign_pieces(tcounts):
    """Assign each class's tiles to (core, block) pieces.

    Pieces are the per-core window blocks with capacities BLK_CAPS. Returns
    per-core, per-block: (class_id or None, [class-tile indices])."""
    nclass = len(tcounts)
    navail = {4: 0, 3: 0, 2: 0}
    for cap in BLK_CAPS:
        navail[cap] += N_CORES
    order = sorted(range(nclass), key=lambda c: -tcounts[c])

    def combos(need, avail):
        out = []
        for n4 in range(avail[4] + 1):
            for n3 in range(avail[3] + 1):
                for n2 in range(avail[2] + 1):
                    tot = 4 * n4 + 3 * n3 + 2 * n2
                    if tot >= need and tot - need <= 3:
                        out.append((tot - need, n4 + n3 + n2, n4, n3, n2))
        out.sort()
        return out

    chosen = [None] * nclass

    def dfs(i, avail):
        if i == len(order):
            return True
        c = order[i]
        for (_ov, _np, n4, n3, n2) in combos(tcounts[c], avail):
            avail2 = {4: avail[4] - n4, 3: avail[3] - n3, 2: avail[2] - n2}
            chosen[c] = (n4, n3, n2)
            if dfs(i + 1, avail2):
                return True
        chosen[c] = None
        return False

    if not dfs(0, dict(navail)):
        raise RuntimeError(f"piece assignment failed for {tcounts}")

    free = {cap: [] for cap in (4, 3, 2)}
    for core in range(N_CORES):
        for blk, cap in enumerate(BLK_CAPS):
            free[cap].append((core, blk))
    plan = [[(None, []) for _ in BLK_CAPS] for _ in range(N_CORES)]
    for c in order:
        n4, n3, n2 = chosen[c]
        pieces = []
        for cap, npc in ((4, n4), (3, n3), (2, n2)):
            for _ in range(npc):
                pieces.append(free[cap].pop(0) + (cap,))
        ti = 0
        for (core, blk, cap) in pieces:
            take = min(cap, tcounts[c] - ti)
            plan[core][blk] = (c, list(range(ti, ti + take)))
            ti += take
        assert ti >= tcounts[c]
    return plan


def _fp8_residual_rows(v):
    """Split v (f64) into 2 fp8 rows r0+r1 ~ v with error ~0.5."""
    import ml_dtypes
    fp8 = ml_dtypes.float8_e4m3
    r0 = np.asarray(v, np.float32).astype(fp8)
    rem = v - r0.astype(np.float64)
    r1 = np.asarray(rem, np.float32).astype(fp8)
    return r0, r1


def _prep_inputs(x, sc, tg):
    import ml_dtypes
    fp8 = ml_dtypes.float8_e4m3

    n, d = x.shape
    perm = np.argsort(tg, kind="stable")
    xs = np.ascontiguousarray(x[perm])
    ss = np.ascontiguousarray(sc[perm])
    ts = tg[perm]
    nclass = int(ts.max()) + 1
    clo = np.searchsorted(ts, np.arange(nclass), "left")
    chi = np.searchsorted(ts, np.arange(nclass), "right")
    widths = (chi - clo).astype(int)
    assert widths.max() <= WT, (widths.max(), WT)
    assert widths.min() >= K + 1, widths.min()
    tcounts = [int(-(-w // 128)) for w in widths]
    assert sum(tcounts) <= N_CORES * TPC

    plan = _assign_pieces(tcounts)
    xsT = np.ascontiguousarray(xs.T)          # (128, N)
    k2 = (xs.astype(np.float64) ** 2).sum(1)  # |x_j|^2 per sorted row
    # pad-column poison (2*FP8MIN) must sit below the row's top-6 P values,
    # which are >= |x_i|^2 - d2(5th NN) >= -max|x|^2 comfortably.
    assert 2 * FP8MIN < -(1.5 * k2.max()) - 50.0, k2.max()

    in_maps = []
    meta = []                                  # per core: list of tile descs
    for core in range(N_CORES):
        keys3 = np.zeros((3, 66, 2, WT), np.float32)
        keys3[:, 64, :, :] = FP8MIN            # bias rows: pad-col poison
        q2t = np.zeros((66, 2, TPC * 128), np.float32)
        q2t[64, :, :] = 1.0                    # all-ones row against bias
        aux = np.zeros((128, TPC * 7 + 2 * TPC), np.float32)
        scoresr = aux[:, :TPC * 7].reshape(128, TPC, 7)
        tqr = aux[:, TPC * 7:TPC * 8]
        flagq = aux[:, TPC * 8:TPC * 9]

        for blk in range(len(BLK_CAPS)):
            c, _tiles = plan[core][blk]
            if c is None:
                continue
            w = widths[c]
            win = xsT[:, clo[c]:chi[c]]                    # (128, w)
            keys3[blk, 0:64, 0, 0:w] = win[0:64]
            keys3[blk, 0:64, 1, 0:w] = win[64:128]
            r0, r1 = _fp8_residual_rows(-k2[clo[c]:chi[c]])
            keys3[blk, 64, 0, 0:w] = r0.astype(np.float32)
            keys3[blk, 64, 1, 0:w] = r1.astype(np.float32)

        tiles = []
        slot = {0: 0, 1: 4, 2: 7}  # first tile slot of each block
        for blk in range(len(BLK_CAPS)):
            c, tlist = plan[core][blk]
            s0 = slot[blk]
            for j, ti in enumerate(tlist):
                t = s0 + j
                qlo = clo[c] + 128 * ti
                qn = int(min(128, chi[c] - qlo))
                qw = 2.0 * xsT[:, qlo:qlo + qn]
                q2t[0:64, 0, t * 128:t * 128 + qn] = qw[0:64]
                q2t[0:64, 1, t * 128:t * 128 + qn] = qw[64:128]
                scoresr[:qn, t, :] = ss[qlo:qlo + qn]
                tqr[:qn, t] = float(c)
                flagq[:qn, t] = 1.0
                tiles.append((t, c, int(qlo), qn))
        meta.append(tiles)

        in_maps.append({
            "keys3": keys3.astype(fp8),
            "q2t": q2t.astype(fp8),
            "aux": np.ascontiguousarray(aux),
        })

    # per-window-block compute width: widest real class in that block slot
    wblk = []
    for b in range(len(BLK_CAPS)):
        w = 16
        for core in range(N_CORES):
            c, _ = plan[core][b]
            if c is not None:
                w = max(w, int(widths[c]))
        wblk.append(min(WT, w))
    return in_maps, meta, tuple(wblk)


def kernel(input, scores, target):
    global LAST_RESULTS
    _maybe_enable_trace_hook()

    x = np.asarray(input, np.float32)
    sc = np.asarray(scores, np.float32)
    tg = np.asarray(target).astype(np.int64)
    n, d = x.shape

    in_maps, _meta, wblk = _prep_inputs(x, sc, tg)

    if wblk not in _PROGRAM_CACHE:
        _PROGRAM_CACHE[wblk] = _build_program(wblk)
    nc = _PROGRAM_CACHE[wblk]

    res = bass_utils.run_bass_kernel_spmd(
        nc, in_maps, core_ids=list(range(N_CORES)))
    LAST_RESULTS = res

    pair_d2 = 0.0
    ce_sum = 0.0
    for r in res.results:
        o = np.asarray(r["out"], np.float64).reshape(-1)
        pair_d2 += o[0]
        ce_sum += o[1]

    loss = ce_sum / n + pair_d2 * 0.5 / (K * d)
    return np.float32(loss)
